# revision 1
# baseline (speedup 1.0000x reference)
"""HashEncoderHyFluid multilevel hash-grid encoding on 8 Trainium2 NeuronCores.

Strategy (data-parallel over the B=131072 points axis, per sharding hint):
  - Each of the 8 cores processes 16384 points against the full table (HBM).
  - Per point-chunk, DVE computes the 16 corner pair-indices for all 16
    levels with level-batched instructions (int32; primes pre-masked to the
    19 low bits that survive the % 2^19, so nothing overflows int32).
  - GPSIMD indirect DMA gathers the (f0, f1) pairs straight from the HBM
    table (one 8-byte descriptor per corner lookup).
  - DVE multiplies by the interpolation weights and reduces over the 16
    corners with a strided tensor_reduce.

All level metadata is derived from the fixed module constants and hardcoded
here (the harness always passes the same shapes/sizes/indicator/offsets).
"""

import functools

import numpy as np

NUM_SCALES = 16
MAX_PARAMS = 2**19
B = 131072
NCORES = 8
BPC = B // NCORES          # 16384 points per core
P = 128                    # SBUF partitions
NPP = BPC // P             # 128 points per partition
CHUNK = 16                 # points per partition per chunk
NCHUNKS = NPP // CHUNK     # 8
MASK19 = (1 << 19) - 1

_MIN_RES = np.array([16, 16, 16, 16], dtype=np.float64)
_MAX_RES = np.array([256, 256, 256, 128], dtype=np.float64)
_PRIMES = np.array([1, 2654435761, 805459861, 3674653429], dtype=np.uint64)


def _level_meta():
    b = np.exp((np.log(_MAX_RES) - np.log(_MIN_RES)) / (NUM_SCALES - 1))
    res, offs2, ind, strides = [], [], [], []
    total = 0
    for s in range(NUM_SCALES):
        r = np.ceil(_MIN_RES * np.power(b, s)).astype(np.int64)
        raw = int((r[0] + 1) * (r[1] + 1) * (r[2] + 1) * (r[3] + 1))
        p = raw if raw % 8 == 0 else (raw + 7) // 8 * 8
        p = min(MAX_PARAMS, p)
        res.append(r)
        ind.append(1 if raw <= p else 0)
        offs2.append(total // 2)
        r1 = r + 1
        strides.append([1, int(r1[0]), int(r1[0] * r1[1]), int(r1[0] * r1[1] * r1[2])])
        total += p * 2
    pm = [int(x & MASK19) for x in _PRIMES]
    return (np.array(res, np.int64), np.array(offs2, np.int64),
            np.array(ind, np.int64), np.array(strides, np.int64), pm, total)


RES, OFFS2, IND, STRIDES, PM, TOTAL = _level_meta()
NPAIRS = TOTAL // 2
L = NUM_SCALES
LU = int(np.sum(IND))          # 3 under (direct-index) levels: 0..LU-1
LF = L - LU                    # 13 fast-hash levels: LU..15


@functools.lru_cache(maxsize=1)
def _build():
    from concourse import bacc, bass, mybir
    import concourse.tile as tile

    f32 = mybir.dt.float32
    i32 = mybir.dt.int32
    OP = mybir.AluOpType

    nc = bacc.Bacc("TRN2", target_bir_lowering=False, debug=False)

    xyz = nc.dram_tensor("xyzts", [BPC, 4], f32, kind="ExternalInput")
    tbl = nc.dram_tensor("tablep", [NPAIRS, 2], f32, kind="ExternalInput")
    cstf = nc.dram_tensor("constf", [P, 4 * L], f32, kind="ExternalInput")
    csti = nc.dram_tensor("consti", [P, 4 * LU + L], i32, kind="ExternalInput")
    outd = nc.dram_tensor("out", [BPC, 32], f32, kind="ExternalOutput")

    n = CHUNK
    with tile.TileContext(nc) as tc:
        with (
            tc.tile_pool(name="io", bufs=1) as io_pool,
            tc.tile_pool(name="coord", bufs=2) as cpool,
            tc.tile_pool(name="wts", bufs=2) as wpool,
            tc.tile_pool(name="hash", bufs=2) as hpool,
            tc.tile_pool(name="gath", bufs=4) as gpool,
            tc.tile_pool(name="red", bufs=3) as rpool,
            tc.tile_pool(name="outp", bufs=2) as opool,
        ):
            xin = io_pool.tile([P, NPP, 4], f32)
            nc.sync.dma_start(out=xin[:], in_=xyz[:].rearrange("(p n) d -> p n d", p=P))
            ctf = io_pool.tile([P, 4 * L], f32)
            nc.sync.dma_start(out=ctf[:], in_=cstf[:])
            cti = io_pool.tile([P, 4 * LU + L], i32)
            nc.sync.dma_start(out=cti[:], in_=csti[:])

            for c in range(NCHUNKS):
                # ---- coordinates / fractions -------------------------------
                pf, gi, fr, om = [], [], [], []
                for d in range(4):
                    x_d = xin[:, c * n:(c + 1) * n, d]                 # [P, n]
                    pf_d = cpool.tile([P, L, n], f32, name=f"pf{d}_{c}", tag=f"pf{d}")
                    nc.vector.tensor_tensor(
                        out=pf_d[:],
                        in0=x_d.unsqueeze(1).broadcast_to([P, L, n]),
                        in1=ctf[:, d * L:(d + 1) * L].unsqueeze(2).broadcast_to([P, L, n]),
                        op=OP.mult)
                    # floor(pos): the HW f32->i32 cast rounds to nearest, so
                    # cast, compare the round-trip against pos, and subtract
                    # the overshoot (exact; all values are small integers).
                    gi_d = cpool.tile([P, L, n], i32, name=f"gi{d}_{c}", tag=f"gi{d}")
                    nc.vector.tensor_copy(out=gi_d[:], in_=pf_d[:])
                    gf_d = cpool.tile([P, L, n], f32, name=f"gf{d}_{c}", tag=f"gf{d}")
                    nc.vector.tensor_copy(out=gf_d[:], in_=gi_d[:])
                    corr = cpool.tile([P, L, n], f32, name=f"corr{d}_{c}", tag=f"corr{d}")
                    nc.vector.tensor_tensor(out=corr[:], in0=gf_d[:], in1=pf_d[:],
                                            op=OP.is_gt)
                    nc.vector.tensor_tensor(out=gf_d[:], in0=gf_d[:], in1=corr[:],
                                            op=OP.subtract)
                    nc.vector.tensor_copy(out=gi_d[:], in_=gf_d[:])
                    fr_d = cpool.tile([P, L, n], f32, name=f"fr{d}_{c}", tag=f"fr{d}")
                    nc.vector.tensor_tensor(out=fr_d[:], in0=pf_d[:], in1=gf_d[:],
                                            op=OP.subtract)
                    om_d = cpool.tile([P, L, n], f32, name=f"om{d}_{c}", tag=f"om{d}")
                    nc.vector.tensor_scalar(out=om_d[:], in0=fr_d[:], scalar1=-1.0,
                                            scalar2=1.0, op0=OP.mult, op1=OP.add)
                    pf.append(pf_d); gi.append(gi_d); fr.append(fr_d); om.append(om_d)

                # ---- interpolation weights (order matches reference) -------
                wxy = wpool.tile([P, 4, L, n], f32, name=f"wxy_{c}", tag="wxy")
                for j01 in range(4):
                    nc.vector.tensor_tensor(
                        out=wxy[:, j01], op=OP.mult,
                        in0=(fr[0] if j01 & 1 else om[0])[:],
                        in1=(fr[1] if j01 & 2 else om[1])[:])
                wxyz = wpool.tile([P, 8, L, n], f32, name=f"wxyz_{c}", tag="wxyz")
                for j012 in range(8):
                    nc.vector.tensor_tensor(
                        out=wxyz[:, j012], op=OP.mult,
                        in0=wxy[:, j012 & 3],
                        in1=(fr[2] if j012 & 4 else om[2])[:])
                w = wpool.tile([P, L, 16, n], f32, name=f"w_{c}", tag="w")
                for j in range(16):
                    nc.vector.tensor_tensor(
                        out=w[:, :, j, :], op=OP.mult,
                        in0=wxyz[:, j & 7],
                        in1=(fr[3] if j & 8 else om[3])[:])

                pit = hpool.tile([P, L, 16, n], i32, name=f"pit_{c}", tag="pit")

                # ---- fast-hash levels (LU..L-1) ----------------------------
                # DVE evaluates int32 mult/add through its fp32 ALU, so every
                # intermediate must stay below 2^24. Split pm = pm_hi*2^10 +
                # pm_lo; with g <= 257 each partial product is exact, and
                # ((g*pm_hi) & 0x1FF) << 10 keeps the recombined sum < 2^19.6
                # while preserving the low 19 bits that survive the final mask.
                a = [gi[0][:, LU:, :]]
                ap_ = []
                a0p = hpool.tile([P, LF, n], i32, name=f"a0p_{c}", tag="a0p")
                nc.vector.tensor_scalar(out=a0p[:], in0=a[0], scalar1=1,
                                        scalar2=None, op0=OP.add)
                ap_.append(a0p[:])
                for d in (1, 2, 3):
                    pm_lo = PM[d] & 0x3FF
                    pm_hi = PM[d] >> 10
                    mlo = hpool.tile([P, LF, n], i32, name=f"mlo{d}_{c}", tag=f"mlo{d}")
                    nc.vector.tensor_scalar(out=mlo[:], in0=gi[d][:, LU:, :],
                                            scalar1=pm_lo, scalar2=None, op0=OP.mult)
                    mhi = hpool.tile([P, LF, n], i32, name=f"mhi{d}_{c}", tag=f"mhi{d}")
                    nc.vector.tensor_scalar(out=mhi[:], in0=gi[d][:, LU:, :],
                                            scalar1=pm_hi, scalar2=None, op0=OP.mult)
                    nc.vector.tensor_scalar(out=mhi[:], in0=mhi[:], scalar1=0x1FF,
                                            scalar2=None, op0=OP.bitwise_and)
                    nc.vector.tensor_scalar(out=mhi[:], in0=mhi[:], scalar1=1024,
                                            scalar2=None, op0=OP.mult)
                    ad = hpool.tile([P, LF, n], i32, name=f"af{d}_{c}", tag=f"af{d}")
                    nc.vector.tensor_tensor(out=ad[:], in0=mlo[:], in1=mhi[:],
                                            op=OP.add)
                    adp = hpool.tile([P, LF, n], i32, name=f"afp{d}_{c}", tag=f"afp{d}")
                    nc.vector.tensor_scalar(out=adp[:], in0=ad[:], scalar1=PM[d] & MASK19,
                                            scalar2=None, op0=OP.add)
                    a.append(ad[:]); ap_.append(adp[:])
                t01 = hpool.tile([P, 4, LF, n], i32, name=f"t01_{c}", tag="t01")
                t23 = hpool.tile([P, 4, LF, n], i32, name=f"t23_{c}", tag="t23")
                for jj in range(4):
                    nc.vector.tensor_tensor(out=t01[:, jj], op=OP.bitwise_xor,
                                            in0=(ap_[0] if jj & 1 else a[0]),
                                            in1=(ap_[1] if jj & 2 else a[1]))
                    nc.vector.tensor_tensor(out=t23[:, jj], op=OP.bitwise_xor,
                                            in0=(ap_[2] if jj & 1 else a[2]),
                                            in1=(ap_[3] if jj & 2 else a[3]))
                for j in range(16):
                    nc.vector.tensor_tensor(out=pit[:, LU:, j, :], op=OP.bitwise_xor,
                                            in0=t01[:, j & 3], in1=t23[:, (j >> 2) & 3])
                nc.vector.tensor_scalar(out=pit[:, LU:, :, :], in0=pit[:, LU:, :, :],
                                        scalar1=MASK19, scalar2=None,
                                        op0=OP.bitwise_and)
                nc.vector.tensor_tensor(
                    out=pit[:, LU:, :, :], op=OP.add, in0=pit[:, LU:, :, :],
                    in1=cti[:, 4 * LU + LU:4 * LU + L]
                        .unsqueeze(2).unsqueeze(3).broadcast_to([P, LF, 16, n]))

                # ---- under (direct-index) levels 0..LU-1 -------------------
                au = [gi[0][:, :LU, :]]
                aup = []
                b0p = hpool.tile([P, LU, n], i32, name=f"b0p_{c}", tag="b0p")
                nc.vector.tensor_scalar(out=b0p[:], in0=au[0], scalar1=1,
                                        scalar2=None, op0=OP.add)
                aup.append(b0p[:])
                for d in (1, 2, 3):
                    sview = cti[:, d * LU:(d + 1) * LU].unsqueeze(2).broadcast_to([P, LU, n])
                    ad = hpool.tile([P, LU, n], i32, name=f"au{d}_{c}", tag=f"au{d}")
                    nc.vector.tensor_tensor(out=ad[:], in0=gi[d][:, :LU, :], in1=sview,
                                            op=OP.mult)
                    adp = hpool.tile([P, LU, n], i32, name=f"aup{d}_{c}", tag=f"aup{d}")
                    nc.vector.tensor_tensor(out=adp[:], in0=ad[:], in1=sview, op=OP.add)
                    au.append(ad[:]); aup.append(adp[:])
                u01 = hpool.tile([P, 4, LU, n], i32, name=f"u01_{c}", tag="u01")
                u23 = hpool.tile([P, 4, LU, n], i32, name=f"u23_{c}", tag="u23")
                for jj in range(4):
                    nc.vector.tensor_tensor(out=u01[:, jj], op=OP.add,
                                            in0=(aup[0] if jj & 1 else au[0]),
                                            in1=(aup[1] if jj & 2 else au[1]))
                    nc.vector.tensor_tensor(out=u23[:, jj], op=OP.add,
                                            in0=(aup[2] if jj & 1 else au[2]),
                                            in1=(aup[3] if jj & 2 else au[3]))
                for j in range(16):
                    nc.vector.tensor_tensor(out=pit[:, :LU, j, :], op=OP.add,
                                            in0=u01[:, j & 3], in1=u23[:, (j >> 2) & 3])
                nc.vector.tensor_tensor(
                    out=pit[:, :LU, :, :], op=OP.add, in0=pit[:, :LU, :, :],
                    in1=cti[:, 4 * LU:4 * LU + LU]
                        .unsqueeze(2).unsqueeze(3).broadcast_to([P, LU, 16, n]))

                # ---- gather + weighted corner reduction --------------------
                ot = opool.tile([P, n, 32], f32, name=f"ot_{c}", tag="ot")
                for l in range(L):
                    gt = gpool.tile([P, 16 * n, 2], f32, name=f"gt_{c}_{l}", tag="gt")
                    # HW walrus only unrolls indirect DMA correctly for a
                    # single offset per partition: issue one DMA per column.
                    pit_flat = pit[:, l, :, :].rearrange("p a b -> p (a b)")
                    for col in range(16 * n):
                        nc.gpsimd.indirect_dma_start(
                            out=gt[:, col, :], out_offset=None, in_=tbl[:],
                            in_offset=bass.IndirectOffsetOnAxis(
                                ap=pit_flat[:, col:col + 1], axis=0))
                    for k in range(2):
                        wf = rpool.tile([P, 16, n], f32, name=f"wf_{c}_{l}_{k}", tag="wf")
                        nc.vector.tensor_tensor(
                            out=wf[:], in0=w[:, l],
                            in1=gt[:, :, k].rearrange("p (a b) -> p a b", a=16),
                            op=OP.mult)
                        nc.vector.tensor_reduce(
                            out=ot[:, :, 2 * l + k], in_=wf[:].transpose([0, 2, 1]),
                            axis=mybir.AxisListType.X, op=OP.add)
                nc.sync.dma_start(
                    out=outd[:].rearrange("(p n) f -> p n f", p=P)[:, c * n:(c + 1) * n, :],
                    in_=ot[:])

    nc.compile()
    return nc


def _const_arrays():
    constf = np.zeros((P, 4 * L), np.float32)
    for d in range(4):
        constf[:, d * L:(d + 1) * L] = RES[:, d].astype(np.float32)
    consti = np.zeros((P, 4 * LU + L), np.int32)
    for d in range(4):
        consti[:, d * LU:(d + 1) * LU] = STRIDES[:LU, d]
    consti[:, 4 * LU:] = OFFS2
    return constf, consti


def _run(xyzts, table, trace=False):
    from concourse import bass_utils

    xyzts = np.ascontiguousarray(np.asarray(xyzts), dtype=np.float32)
    tablep = np.ascontiguousarray(np.asarray(table), dtype=np.float32).reshape(NPAIRS, 2)
    constf, consti = _const_arrays()
    nc = _build()
    in_maps = [
        {"xyzts": xyzts[c * BPC:(c + 1) * BPC], "tablep": tablep,
         "constf": constf, "consti": consti}
        for c in range(NCORES)
    ]
    res = bass_utils.run_bass_kernel_spmd(
        nc, in_maps, core_ids=list(range(NCORES)), trace=trace)
    out = np.concatenate([res.results[c]["out"] for c in range(NCORES)], axis=0)
    return out, res


def bench(xyzts, table, iters=3):
    """Steady-state wall time per execution with device-resident inputs.

    Mirrors bass2jax.run_bass_via_pjrt's shard_map jit, but keeps the input
    arrays on device across calls so the measured time is the NEFF execution
    (plus dispatch), not the ~0.5GB host staging.
    """
    import time
    import jax
    from jax.sharding import Mesh, PartitionSpec
    from jax.experimental.shard_map import shard_map
    from concourse import bass2jax, mybir

    nc = _build()
    bass2jax.install_neuronx_cc_hook()
    xyzts = np.ascontiguousarray(np.asarray(xyzts), dtype=np.float32)
    tablep = np.ascontiguousarray(np.asarray(table), dtype=np.float32).reshape(NPAIRS, 2)
    constf, consti = _const_arrays()

    partition_name = nc.partition_id_tensor.name if nc.partition_id_tensor else None
    in_names, out_names, out_avals, zero_outs = [], [], [], []
    for alloc in nc.m.functions[0].allocations:
        if not isinstance(alloc, mybir.MemoryLocationSet):
            continue
        name = alloc.memorylocations[0].name
        if alloc.kind == "ExternalInput":
            if name != partition_name:
                in_names.append(name)
        elif alloc.kind == "ExternalOutput":
            out_names.append(name)
            shape = tuple(alloc.tensor_shape)
            dtype = mybir.dt.np(alloc.dtype)
            out_avals.append(jax.core.ShapedArray(shape, dtype))
            zero_outs.append(np.zeros(shape, dtype))
    n_params = len(in_names)
    all_names = in_names + out_names
    if partition_name is not None:
        all_names = all_names + [partition_name]

    def _body(*args):
        operands = list(args)
        if partition_name is not None:
            operands.append(bass2jax.partition_id_tensor())
        return tuple(bass2jax._bass_exec_p.bind(
            *operands, out_avals=tuple(out_avals), in_names=tuple(all_names),
            out_names=tuple(out_names), lowering_input_output_aliases=(),
            sim_require_finite=True, sim_require_nnan=True, nc=nc))

    devices = jax.devices()[:NCORES]
    mesh = Mesh(np.asarray(devices), ("core",))
    jitted = jax.jit(shard_map(
        _body, mesh=mesh,
        in_specs=(PartitionSpec("core"),) * (n_params + len(out_names)),
        out_specs=(PartitionSpec("core"),) * len(out_names)), keep_unused=True)

    per_core = {"xyzts": [xyzts[c * BPC:(c + 1) * BPC] for c in range(NCORES)],
                "tablep": [tablep] * NCORES, "constf": [constf] * NCORES,
                "consti": [consti] * NCORES}
    dev_in = [jax.device_put(np.concatenate(per_core[n], axis=0)) for n in in_names]
    dev_zero = [jax.device_put(np.concatenate([z] * NCORES, axis=0)) for z in zero_outs]
    outs = jitted(*dev_in, *dev_zero)
    jax.block_until_ready(outs)
    times = []
    for _ in range(iters):
        t0 = time.perf_counter()
        outs = jitted(*dev_in, *dev_zero)
        jax.block_until_ready(outs)
        times.append(time.perf_counter() - t0)
    out = np.asarray(outs[0])
    return out, min(times)


def kernel(**inputs):
    out, _ = _run(inputs["xyzts"], inputs["table"])
    return out



# revision 4
# speedup vs baseline: 1.1434x; 1.1434x over previous
"""HashEncoderHyFluid multilevel hash-grid encoding on 8 Trainium2 NeuronCores.

Strategy (data-parallel over the B=131072 points axis, per sharding hint):
  - Each of the 8 cores processes 16384 points against the full table (HBM).
  - Per point-chunk, DVE computes the 16 corner pair-indices for all 16
    levels with level-batched instructions (int32; primes pre-masked to the
    19 low bits that survive the % 2^19, so nothing overflows int32).
  - GPSIMD indirect DMA gathers the (f0, f1) pairs straight from the HBM
    table. TRN2's indirect DMA issues exactly one descriptor per partition
    per instruction, so a chunk needs L*16*n = 4096 instructions; they are
    issued back-to-back into one large SBUF tile so the gpsimd engine runs
    at its ~0.7us/instruction issue rate instead of stalling on consumers.
  - DVE multiplies by the interpolation weights and reduces over the 16
    corners with a strided tensor_reduce, one chunk behind the gather
    stream (software pipelining via double-buffered pit/gt/w tiles).

All level metadata is derived from the fixed module constants and hardcoded
here (the harness always passes the same shapes/sizes/indicator/offsets).
"""

import functools

import numpy as np

NUM_SCALES = 16
MAX_PARAMS = 2**19
B = 131072
NCORES = 8
BPC = B // NCORES          # 16384 points per core
P = 128                    # SBUF partitions
NPP = BPC // P             # 128 points per partition
CHUNK = 16                 # points per partition per chunk
NCHUNKS = NPP // CHUNK     # 8
MASK19 = (1 << 19) - 1

_MIN_RES = np.array([16, 16, 16, 16], dtype=np.float64)
_MAX_RES = np.array([256, 256, 256, 128], dtype=np.float64)
_PRIMES = np.array([1, 2654435761, 805459861, 3674653429], dtype=np.uint64)


def _level_meta():
    b = np.exp((np.log(_MAX_RES) - np.log(_MIN_RES)) / (NUM_SCALES - 1))
    res, offs2, ind, strides = [], [], [], []
    total = 0
    for s in range(NUM_SCALES):
        r = np.ceil(_MIN_RES * np.power(b, s)).astype(np.int64)
        raw = int((r[0] + 1) * (r[1] + 1) * (r[2] + 1) * (r[3] + 1))
        p = raw if raw % 8 == 0 else (raw + 7) // 8 * 8
        p = min(MAX_PARAMS, p)
        res.append(r)
        ind.append(1 if raw <= p else 0)
        offs2.append(total // 2)
        r1 = r + 1
        strides.append([1, int(r1[0]), int(r1[0] * r1[1]), int(r1[0] * r1[1] * r1[2])])
        total += p * 2
    pm = [int(x & MASK19) for x in _PRIMES]
    return (np.array(res, np.int64), np.array(offs2, np.int64),
            np.array(ind, np.int64), np.array(strides, np.int64), pm, total)


RES, OFFS2, IND, STRIDES, PM, TOTAL = _level_meta()
NPAIRS = TOTAL // 2
L = NUM_SCALES
LU = int(np.sum(IND))          # 3 under (direct-index) levels: 0..LU-1
LF = L - LU                    # 13 fast-hash levels: LU..15


@functools.lru_cache(maxsize=1)
def _build():
    from concourse import bacc, bass, mybir
    import concourse.tile as tile

    f32 = mybir.dt.float32
    i32 = mybir.dt.int32
    OP = mybir.AluOpType

    nc = bacc.Bacc("TRN2", target_bir_lowering=False, debug=False)

    xyz = nc.dram_tensor("xyzts", [BPC, 4], f32, kind="ExternalInput")
    tbl = nc.dram_tensor("tablep", [NPAIRS, 2], f32, kind="ExternalInput")
    cstf = nc.dram_tensor("constf", [P, 4 * L], f32, kind="ExternalInput")
    csti = nc.dram_tensor("consti", [P, 4 * LU + L], i32, kind="ExternalInput")
    outd = nc.dram_tensor("out", [BPC, 32], f32, kind="ExternalOutput")

    n = CHUNK
    NC16 = L * 16 * n          # 4096 corner lookups per partition per chunk

    with tile.TileContext(nc) as tc:
        with (
            tc.tile_pool(name="io", bufs=1) as io_pool,
            # DVE-internal intermediates: consumed in program order on the
            # same engine, so one buffer suffices (no cross-engine overlap).
            tc.tile_pool(name="coord", bufs=1) as cpool,
            tc.tile_pool(name="wtmp", bufs=1) as wtpool,
            tc.tile_pool(name="hash", bufs=1) as hpool,
            # Cross-engine tiles: double-buffered for the chunk pipeline.
            tc.tile_pool(name="wts", bufs=2) as wpool,
            tc.tile_pool(name="pitp", bufs=2) as ppool,
            tc.tile_pool(name="gath", bufs=2) as gpool,
            tc.tile_pool(name="red", bufs=2) as rpool,
            tc.tile_pool(name="outp", bufs=2) as opool,
        ):
            xin = io_pool.tile([P, NPP, 4], f32)
            nc.sync.dma_start(out=xin[:], in_=xyz[:].rearrange("(p n) d -> p n d", p=P))
            ctf = io_pool.tile([P, 4 * L], f32)
            nc.sync.dma_start(out=ctf[:], in_=cstf[:])
            cti = io_pool.tile([P, 4 * LU + L], i32)
            nc.sync.dma_start(out=cti[:], in_=csti[:])

            def emit_pit(c):
                """DVE: coordinates, weights, and corner pair-indices."""
                pf, gi, fr, om = [], [], [], []
                for d in range(4):
                    x_d = xin[:, c * n:(c + 1) * n, d]                 # [P, n]
                    pf_d = cpool.tile([P, L, n], f32, name=f"pf{d}_{c}", tag=f"pf{d}")
                    nc.vector.tensor_tensor(
                        out=pf_d[:],
                        in0=x_d.unsqueeze(1).broadcast_to([P, L, n]),
                        in1=ctf[:, d * L:(d + 1) * L].unsqueeze(2).broadcast_to([P, L, n]),
                        op=OP.mult)
                    # floor(pos): the HW f32->i32 cast rounds to nearest, so
                    # cast, compare the round-trip against pos, and subtract
                    # the overshoot (exact; all values are small integers).
                    gi_d = cpool.tile([P, L, n], i32, name=f"gi{d}_{c}", tag=f"gi{d}")
                    nc.vector.tensor_copy(out=gi_d[:], in_=pf_d[:])
                    gf_d = cpool.tile([P, L, n], f32, name=f"gf{d}_{c}", tag=f"gf{d}")
                    nc.vector.tensor_copy(out=gf_d[:], in_=gi_d[:])
                    corr = cpool.tile([P, L, n], f32, name=f"corr{d}_{c}", tag=f"corr{d}")
                    nc.vector.tensor_tensor(out=corr[:], in0=gf_d[:], in1=pf_d[:],
                                            op=OP.is_gt)
                    nc.vector.tensor_tensor(out=gf_d[:], in0=gf_d[:], in1=corr[:],
                                            op=OP.subtract)
                    nc.vector.tensor_copy(out=gi_d[:], in_=gf_d[:])
                    fr_d = cpool.tile([P, L, n], f32, name=f"fr{d}_{c}", tag=f"fr{d}")
                    nc.vector.tensor_tensor(out=fr_d[:], in0=pf_d[:], in1=gf_d[:],
                                            op=OP.subtract)
                    om_d = cpool.tile([P, L, n], f32, name=f"om{d}_{c}", tag=f"om{d}")
                    nc.vector.tensor_scalar(out=om_d[:], in0=fr_d[:], scalar1=-1.0,
                                            scalar2=1.0, op0=OP.mult, op1=OP.add)
                    pf.append(pf_d); gi.append(gi_d); fr.append(fr_d); om.append(om_d)

                # ---- interpolation weights (order matches reference) -------
                wxy = wtpool.tile([P, 4, L, n], f32, name=f"wxy_{c}", tag="wxy")
                for j01 in range(4):
                    nc.vector.tensor_tensor(
                        out=wxy[:, j01], op=OP.mult,
                        in0=(fr[0] if j01 & 1 else om[0])[:],
                        in1=(fr[1] if j01 & 2 else om[1])[:])
                wxyz = wtpool.tile([P, 8, L, n], f32, name=f"wxyz_{c}", tag="wxyz")
                for j012 in range(8):
                    nc.vector.tensor_tensor(
                        out=wxyz[:, j012], op=OP.mult,
                        in0=wxy[:, j012 & 3],
                        in1=(fr[2] if j012 & 4 else om[2])[:])
                w = wpool.tile([P, L, 16, n], f32, name=f"w_{c}", tag="w")
                for j in range(16):
                    nc.vector.tensor_tensor(
                        out=w[:, :, j, :], op=OP.mult,
                        in0=wxyz[:, j & 7],
                        in1=(fr[3] if j & 8 else om[3])[:])

                pit = ppool.tile([P, L, 16, n], i32, name=f"pit_{c}", tag="pit")

                # ---- fast-hash levels (LU..L-1) ----------------------------
                # DVE evaluates int32 mult/add through its fp32 ALU, so every
                # intermediate must stay below 2^24. Split pm = pm_hi*2^10 +
                # pm_lo; with g <= 257 each partial product is exact, and
                # ((g*pm_hi) & 0x1FF) << 10 keeps the recombined sum < 2^19.6
                # while preserving the low 19 bits that survive the final mask.
                a = [gi[0][:, LU:, :]]
                ap_ = []
                a0p = hpool.tile([P, LF, n], i32, name=f"a0p_{c}", tag="a0p")
                nc.vector.tensor_scalar(out=a0p[:], in0=a[0], scalar1=1,
                                        scalar2=None, op0=OP.add)
                ap_.append(a0p[:])
                for d in (1, 2, 3):
                    pm_lo = PM[d] & 0x3FF
                    pm_hi = PM[d] >> 10
                    mlo = hpool.tile([P, LF, n], i32, name=f"mlo{d}_{c}", tag=f"mlo{d}")
                    nc.vector.tensor_scalar(out=mlo[:], in0=gi[d][:, LU:, :],
                                            scalar1=pm_lo, scalar2=None, op0=OP.mult)
                    mhi = hpool.tile([P, LF, n], i32, name=f"mhi{d}_{c}", tag=f"mhi{d}")
                    nc.vector.tensor_scalar(out=mhi[:], in0=gi[d][:, LU:, :],
                                            scalar1=pm_hi, scalar2=None, op0=OP.mult)
                    nc.vector.tensor_scalar(out=mhi[:], in0=mhi[:], scalar1=0x1FF,
                                            scalar2=None, op0=OP.bitwise_and)
                    nc.vector.tensor_scalar(out=mhi[:], in0=mhi[:], scalar1=1024,
                                            scalar2=None, op0=OP.mult)
                    ad = hpool.tile([P, LF, n], i32, name=f"af{d}_{c}", tag=f"af{d}")
                    nc.vector.tensor_tensor(out=ad[:], in0=mlo[:], in1=mhi[:],
                                            op=OP.add)
                    adp = hpool.tile([P, LF, n], i32, name=f"afp{d}_{c}", tag=f"afp{d}")
                    nc.vector.tensor_scalar(out=adp[:], in0=ad[:], scalar1=PM[d] & MASK19,
                                            scalar2=None, op0=OP.add)
                    a.append(ad[:]); ap_.append(adp[:])
                t01 = hpool.tile([P, 4, LF, n], i32, name=f"t01_{c}", tag="t01")
                t23 = hpool.tile([P, 4, LF, n], i32, name=f"t23_{c}", tag="t23")
                for jj in range(4):
                    nc.vector.tensor_tensor(out=t01[:, jj], op=OP.bitwise_xor,
                                            in0=(ap_[0] if jj & 1 else a[0]),
                                            in1=(ap_[1] if jj & 2 else a[1]))
                    nc.vector.tensor_tensor(out=t23[:, jj], op=OP.bitwise_xor,
                                            in0=(ap_[2] if jj & 1 else a[2]),
                                            in1=(ap_[3] if jj & 2 else a[3]))
                for j in range(16):
                    nc.vector.tensor_tensor(out=pit[:, LU:, j, :], op=OP.bitwise_xor,
                                            in0=t01[:, j & 3], in1=t23[:, (j >> 2) & 3])
                nc.vector.tensor_scalar(out=pit[:, LU:, :, :], in0=pit[:, LU:, :, :],
                                        scalar1=MASK19, scalar2=None,
                                        op0=OP.bitwise_and)
                nc.vector.tensor_tensor(
                    out=pit[:, LU:, :, :], op=OP.add, in0=pit[:, LU:, :, :],
                    in1=cti[:, 4 * LU + LU:4 * LU + L]
                        .unsqueeze(2).unsqueeze(3).broadcast_to([P, LF, 16, n]))

                # ---- under (direct-index) levels 0..LU-1 -------------------
                au = [gi[0][:, :LU, :]]
                aup = []
                b0p = hpool.tile([P, LU, n], i32, name=f"b0p_{c}", tag="b0p")
                nc.vector.tensor_scalar(out=b0p[:], in0=au[0], scalar1=1,
                                        scalar2=None, op0=OP.add)
                aup.append(b0p[:])
                for d in (1, 2, 3):
                    sview = cti[:, d * LU:(d + 1) * LU].unsqueeze(2).broadcast_to([P, LU, n])
                    ad = hpool.tile([P, LU, n], i32, name=f"au{d}_{c}", tag=f"au{d}")
                    nc.vector.tensor_tensor(out=ad[:], in0=gi[d][:, :LU, :], in1=sview,
                                            op=OP.mult)
                    adp = hpool.tile([P, LU, n], i32, name=f"aup{d}_{c}", tag=f"aup{d}")
                    nc.vector.tensor_tensor(out=adp[:], in0=ad[:], in1=sview, op=OP.add)
                    au.append(ad[:]); aup.append(adp[:])
                u01 = hpool.tile([P, 4, LU, n], i32, name=f"u01_{c}", tag="u01")
                u23 = hpool.tile([P, 4, LU, n], i32, name=f"u23_{c}", tag="u23")
                for jj in range(4):
                    nc.vector.tensor_tensor(out=u01[:, jj], op=OP.add,
                                            in0=(aup[0] if jj & 1 else au[0]),
                                            in1=(aup[1] if jj & 2 else au[1]))
                    nc.vector.tensor_tensor(out=u23[:, jj], op=OP.add,
                                            in0=(aup[2] if jj & 1 else au[2]),
                                            in1=(aup[3] if jj & 2 else au[3]))
                for j in range(16):
                    nc.vector.tensor_tensor(out=pit[:, :LU, j, :], op=OP.add,
                                            in0=u01[:, j & 3], in1=u23[:, (j >> 2) & 3])
                nc.vector.tensor_tensor(
                    out=pit[:, :LU, :, :], op=OP.add, in0=pit[:, :LU, :, :],
                    in1=cti[:, 4 * LU:4 * LU + LU]
                        .unsqueeze(2).unsqueeze(3).broadcast_to([P, LU, 16, n]))
                return pit, w

            def emit_gather(c, pit):
                """GPSIMD: all 4096 corner gathers of the chunk back-to-back."""
                gt = gpool.tile([P, NC16, 2], f32, name=f"gt_{c}", tag="gt")
                pit_flat = pit[:].rearrange("p l j i -> p (l j i)")
                for col in range(NC16):
                    nc.gpsimd.indirect_dma_start(
                        out=gt[:, col, :], out_offset=None, in_=tbl[:],
                        in_offset=bass.IndirectOffsetOnAxis(
                            ap=pit_flat[:, col:col + 1], axis=0))
                return gt

            def emit_consume(c, w, gt):
                """DVE: weighted corner reduction; DMA the chunk's output."""
                ot = opool.tile([P, n, 32], f32, name=f"ot_{c}", tag="ot")
                gtv = gt[:].rearrange("p (l j i) k -> p l j i k", l=L, j=16)
                for l in range(L):
                    for k in range(2):
                        wf = rpool.tile([P, 16, n], f32, name=f"wf_{c}_{l}_{k}", tag="wf")
                        nc.vector.tensor_tensor(
                            out=wf[:], in0=w[:, l], in1=gtv[:, l, :, :, k],
                            op=OP.mult)
                        nc.vector.tensor_reduce(
                            out=ot[:, :, 2 * l + k], in_=wf[:].transpose([0, 2, 1]),
                            axis=mybir.AxisListType.X, op=OP.add)
                nc.sync.dma_start(
                    out=outd[:].rearrange("(p n) f -> p n f", p=P)[:, c * n:(c + 1) * n, :],
                    in_=ot[:])

            # Software pipeline: pit(c+1) is emitted before consume(c) so the
            # DVE works ahead while the gpsimd engine streams chunk c's
            # gathers, and the gather stream never waits on a consumer.
            pit0, w0 = emit_pit(0)
            prev = (0, w0, emit_gather(0, pit0))
            for c in range(1, NCHUNKS):
                pit_c, w_c = emit_pit(c)
                gt_c = emit_gather(c, pit_c)
                emit_consume(prev[0], prev[1], prev[2])
                prev = (c, w_c, gt_c)
            emit_consume(prev[0], prev[1], prev[2])

    nc.compile()
    return nc


def _const_arrays():
    constf = np.zeros((P, 4 * L), np.float32)
    for d in range(4):
        constf[:, d * L:(d + 1) * L] = RES[:, d].astype(np.float32)
    consti = np.zeros((P, 4 * LU + L), np.int32)
    for d in range(4):
        consti[:, d * LU:(d + 1) * LU] = STRIDES[:LU, d]
    consti[:, 4 * LU:] = OFFS2
    return constf, consti


def _run(xyzts, table, trace=False):
    from concourse import bass_utils

    xyzts = np.ascontiguousarray(np.asarray(xyzts), dtype=np.float32)
    tablep = np.ascontiguousarray(np.asarray(table), dtype=np.float32).reshape(NPAIRS, 2)
    constf, consti = _const_arrays()
    nc = _build()
    in_maps = [
        {"xyzts": xyzts[c * BPC:(c + 1) * BPC], "tablep": tablep,
         "constf": constf, "consti": consti}
        for c in range(NCORES)
    ]
    res = bass_utils.run_bass_kernel_spmd(
        nc, in_maps, core_ids=list(range(NCORES)), trace=trace)
    out = np.concatenate([res.results[c]["out"] for c in range(NCORES)], axis=0)
    return out, res


def bench(xyzts, table, iters=3):
    """Steady-state wall time per execution with device-resident inputs.

    Mirrors bass2jax.run_bass_via_pjrt's shard_map jit, but keeps the input
    arrays on device across calls so the measured time is the NEFF execution
    (plus dispatch), not the ~0.5GB host staging.
    """
    import time
    import jax
    from jax.sharding import Mesh, PartitionSpec
    from jax.experimental.shard_map import shard_map
    from concourse import bass2jax, mybir

    nc = _build()
    bass2jax.install_neuronx_cc_hook()
    xyzts = np.ascontiguousarray(np.asarray(xyzts), dtype=np.float32)
    tablep = np.ascontiguousarray(np.asarray(table), dtype=np.float32).reshape(NPAIRS, 2)
    constf, consti = _const_arrays()

    partition_name = nc.partition_id_tensor.name if nc.partition_id_tensor else None
    in_names, out_names, out_avals, zero_outs = [], [], [], []
    for alloc in nc.m.functions[0].allocations:
        if not isinstance(alloc, mybir.MemoryLocationSet):
            continue
        name = alloc.memorylocations[0].name
        if alloc.kind == "ExternalInput":
            if name != partition_name:
                in_names.append(name)
        elif alloc.kind == "ExternalOutput":
            out_names.append(name)
            shape = tuple(alloc.tensor_shape)
            dtype = mybir.dt.np(alloc.dtype)
            out_avals.append(jax.core.ShapedArray(shape, dtype))
            zero_outs.append(np.zeros(shape, dtype))
    n_params = len(in_names)
    all_names = in_names + out_names
    if partition_name is not None:
        all_names = all_names + [partition_name]

    def _body(*args):
        operands = list(args)
        if partition_name is not None:
            operands.append(bass2jax.partition_id_tensor())
        return tuple(bass2jax._bass_exec_p.bind(
            *operands, out_avals=tuple(out_avals), in_names=tuple(all_names),
            out_names=tuple(out_names), lowering_input_output_aliases=(),
            sim_require_finite=True, sim_require_nnan=True, nc=nc))

    devices = jax.devices()[:NCORES]
    mesh = Mesh(np.asarray(devices), ("core",))
    jitted = jax.jit(shard_map(
        _body, mesh=mesh,
        in_specs=(PartitionSpec("core"),) * (n_params + len(out_names)),
        out_specs=(PartitionSpec("core"),) * len(out_names)), keep_unused=True)

    per_core = {"xyzts": [xyzts[c * BPC:(c + 1) * BPC] for c in range(NCORES)],
                "tablep": [tablep] * NCORES, "constf": [constf] * NCORES,
                "consti": [consti] * NCORES}
    dev_in = [jax.device_put(np.concatenate(per_core[n], axis=0)) for n in in_names]
    dev_zero = [jax.device_put(np.concatenate([z] * NCORES, axis=0)) for z in zero_outs]
    outs = jitted(*dev_in, *dev_zero)
    jax.block_until_ready(outs)
    times = []
    for _ in range(iters):
        t0 = time.perf_counter()
        outs = jitted(*dev_in, *dev_zero)
        jax.block_until_ready(outs)
        times.append(time.perf_counter() - t0)
    out = np.asarray(outs[0])
    return out, min(times)


def kernel(**inputs):
    out, _ = _run(inputs["xyzts"], inputs["table"])
    return out
